# revision 23
# baseline (speedup 1.0000x reference)
"""DisplaceChannel (integer displace + per-position 5x5 gaussian depthwise
conv) as a Bass/Tile kernel for 8 Trainium2 NeuronCores.

Math: the 5x5 gaussian kernel is separable and its normalizer factorizes;
the integer shift + 'same' zero-padding fold into banded 64x64 row/col
operators built host-side from the tiny (48,2) `offset`.  Per image:

    out = Vy @ X @ Vx^T

On device each channel-pair of images (same channel, adjacent batches)
runs two chained PE matmuls (fp16 operands, fp32 PSUM accumulate):

  pass1: two concurrent 64x64 quadrant matmuls (tile_position (0,0) and
         (64,64)): lhsT = image rows (stationary), rhs = [R1; R1] slices,
         R1 = Vy^T -> PSUM [128,64] = [A^T R1 ; B^T R1]  ([x', y] each)
  pass2: lhsT = two pass1 pair-results side by side [128,128] (cast fp16),
         rhs = blockdiag(R2, R2) [128,128], R2 = Vx^T
         -> PSUM [128,128]: partitions = (channel-sub, y), cols = (batch, x)

Sharding: data-parallel over batch (4 per core); operators replicated.

I/O layout: the host packs the input (fp16 cast + index permutation) and
unpacks the output so that every device DMA moves contiguous 12KB runs
per partition.  In the natural NCHW layout each (channel,row) is a
256-byte descriptor and the kernel is SDMA-packet-rate-bound; packed, it
runs at the HBM byte roofline.
"""

import numpy as np

from concourse import bacc, mybir, tile
from concourse.bass_utils import run_bass_kernel_spmd

# problem constants (hardcoded per harness contract)
B_FULL, C, H, W = 32, 384, 64, 64
N_CORES = 8
B_LOC = B_FULL // N_CORES          # 4 batches per core
P_POS = 48                         # offset positions; C // P_POS = 8 chan/pos
GROUP = C // P_POS                 # 8 channels share one operator pair
KSZ, SIGMA, CK = 5, 0.5, 2

N_BPAIR = B_LOC // 2               # batch-pairs (2bp, 2bp+1) per core
IN_GROUPS = 12                     # groups per input chunk (96 channels)
OUT_GROUPS = 6                     # groups per output chunk (48 channels)
IN_COLS = IN_GROUPS * GROUP * 64   # 6144 fp16 cols per in-chunk
OUT_COLS = OUT_GROUPS * GROUP * 64  # 3072 fp32 cols per out-chunk
XCOLS = C * 64                     # 24576 per-bp packed cols

FP16 = mybir.dt.float16
FP32 = mybir.dt.float32

_LAST_RESULT = None                # test.py introspection (profile/exec time)


def _shift_conv_matrix(sub, d):
    """[64(src), 64(out)] with R[src,out] = k[i], src = out + i - 2 - d,
    masked by conv zero-pad (0<=out+i-2<64) and shift zero-fill (0<=src<64)."""
    k = np.exp(-((np.arange(KSZ) - CK + sub) ** 2) / (2.0 * SIGMA**2))
    k = k / k.sum()
    R = np.zeros((H, H), dtype=np.float64)
    out = np.arange(H)
    for i in range(KSZ):
        t = out + i - CK            # coordinate in the shifted image
        src = t - d
        m = (t >= 0) & (t < H) & (src >= 0) & (src < H)
        R[src[m], out[m]] += k[i]
    return R


def _build_ops(offset):
    """ops1 [128, 48*64] fp16 : per position the stacked [R1; R1] (R1 = Vy^T)
    ops2 [128, 48*128] fp16 : per position blockdiag(R2, R2)   (R2 = Vx^T)"""
    off_round = np.round(offset.astype(np.float64))
    off_int = off_round.astype(np.int64)
    sub = offset.astype(np.float64) - off_round
    ops1 = np.zeros((128, P_POS * 64), dtype=np.float64)
    ops2 = np.zeros((128, P_POS * 128), dtype=np.float64)
    for p in range(P_POS):
        R1 = _shift_conv_matrix(sub[p, 1], off_int[p, 1])   # y: suby, dy
        R2 = _shift_conv_matrix(sub[p, 0], off_int[p, 0])   # x: subx, dx
        ops1[0:64, 64 * p:64 * p + 64] = R1
        ops1[64:128, 64 * p:64 * p + 64] = R1
        ops2[0:64, 128 * p:128 * p + 64] = R2
        ops2[64:128, 128 * p + 64:128 * p + 128] = R2
    return ops1.astype(np.float16), ops2.astype(np.float16)


def _build_bass():
    nc = bacc.Bacc(
        "TRN2",
        target_bir_lowering=False,
        debug=False,
        num_devices=N_CORES,
    )
    # packed fp16 input: per bp a [128, 24576] block; channel c at cols
    # 64c:64c+64, batch 2bp rows on partitions 0:64, batch 2bp+1 on 64:128.
    x_in = nc.declare_dram_parameter("x", [N_BPAIR, 128, XCOLS], FP16,
                                     isOutput=False)
    ops1_in = nc.declare_dram_parameter("ops1", [128, P_POS * 64], FP16,
                                        isOutput=False)
    ops2_in = nc.declare_dram_parameter("ops2", [128, P_POS * 128], FP16,
                                        isOutput=False)
    # packed output: per bp [128, 24576] f32; cols (g, m, s, x), partitions
    # (u', y); channel = 8g + 2m + u', batch = 2bp + s.
    y_out = nc.declare_dram_parameter("y", [N_BPAIR, 128, XCOLS], FP32,
                                      isOutput=True)

    with tile.TileContext(nc) as tc:
        with (
            tc.tile_pool(name="consts", bufs=1) as consts,
            tc.tile_pool(name="wchunk", bufs=4) as wpool,
            tc.tile_pool(name="l2", bufs=4) as l2pool,
            tc.tile_pool(name="outs", bufs=3) as outpool,
            tc.tile_pool(name="psum1", bufs=3, space="PSUM") as psum1p,
            tc.tile_pool(name="psum2", bufs=3, space="PSUM") as psum2p,
        ):
            t_ops1 = consts.tile([128, P_POS * 64], FP16)
            t_ops2 = consts.tile([128, P_POS * 128], FP16)
            nc.sync.dma_start(out=t_ops1[:], in_=ops1_in[:])
            nc.sync.dma_start(out=t_ops2[:], in_=ops2_in[:])

            # flattened (bp, group) stream with lookahead in-DMA emission
            # (SWDGE executes its queue in emission order, so chunk k+2's
            # DMA is emitted before chunk k's compute).
            chunk_list = []                     # (bp, ga, gb)
            for bp in range(N_BPAIR):
                starts = ([0, 2, 6] if bp == 0 else [0, 6]) + \
                    list(range(12, P_POS, 6))
                for ci, a in enumerate(starts):
                    b = starts[ci + 1] if ci + 1 < len(starts) else P_POS
                    chunk_list.append((bp, a, b))
            g2chunk = {}
            for ci, (bp, a, b) in enumerate(chunk_list):
                for g in range(a, b):
                    g2chunk[(bp, g)] = ci
            LOOKAHEAD = 2
            wts = {}
            n_emitted = 0

            def emit_in_dma(ci):
                bp, a, b = chunk_list[ci]
                wt = wpool.tile([128, (b - a) * GROUP * 64], FP16,
                                tag=f"w{b - a}")
                nc.gpsimd.dma_start(
                    out=wt[:],
                    in_=x_in[bp][:, a * GROUP * 64:b * GROUP * 64])
                wts[ci] = wt

            it = 0
            outs = None
            for bp in range(N_BPAIR):
                for g in range(P_POS):                      # 48 groups / bp
                    ci = g2chunk[(bp, g)]
                    while n_emitted <= min(ci + LOOKAHEAD, len(chunk_list) - 1):
                        emit_in_dma(n_emitted)
                        n_emitted += 1
                    if g % OUT_GROUPS == 0:
                        outs = outpool.tile([128, OUT_COLS], FP32)
                    wt = wts[ci]
                    goff = g - chunk_list[ci][1]     # group offset in chunk

                    rhs1 = t_ops1[:, 64 * g:64 * g + 64]
                    rhs2 = t_ops2[:, 128 * g:128 * g + 128]
                    ps1 = psum1p.tile([128, 512], FP32)
                    ps2 = psum2p.tile([128, 512], FP32)
                    l2 = l2pool.tile([128, 512], FP16)

                    for j in range(GROUP):                   # 8 pairs
                        jc = goff * GROUP + j                # chan in chunk
                        cs = slice(64 * jc, 64 * jc + 64)
                        # A (batch 2bp): quadrant rows 0:63 x cols 0:63
                        nc.tensor.matmul(ps1[0:64, 64 * j:64 * j + 64],
                                         wt[0:64, cs], rhs1[0:64, :],
                                         start=True, stop=True,
                                         tile_position=(0, 0))
                        # B (batch 2bp+1): quadrant rows 64:127 x 64:127
                        nc.tensor.matmul(ps1[64:128, 64 * j:64 * j + 64],
                                         wt[64:128, cs], rhs1[64:128, :],
                                         start=True, stop=True,
                                         tile_position=(64, 64))
                    # psum fp32 -> sbuf fp16 (pass2 stationary operand)
                    if it % 2 == 0:
                        nc.vector.tensor_copy(l2[:], ps1[:])
                    else:
                        nc.scalar.copy(l2[:], ps1[:])
                    for m in range(GROUP // 2):              # 4 two-pair MMs
                        lhsT2 = l2[:, 128 * m:128 * m + 128]
                        nc.tensor.matmul(ps2[:, 128 * m:128 * m + 128],
                                         lhsT2, rhs2,
                                         start=True, stop=True)
                    # final psum -> staging (fp32, full partitions)
                    od = outs[:, 512 * (g % OUT_GROUPS):
                              512 * (g % OUT_GROUPS) + 512]
                    if it % 2 == 0:
                        nc.scalar.copy(od, ps2[:])
                    else:
                        nc.vector.tensor_copy(od, ps2[:])
                    it += 1

                    if g % OUT_GROUPS == OUT_GROUPS - 1:
                        oc = g // OUT_GROUPS
                        nc.sync.dma_start(
                            out=y_out[bp][:, OUT_COLS * oc:OUT_COLS * (oc + 1)],
                            in_=outs[:])
    nc.compile()
    return nc


_NC_CACHE = None


def kernel(x: np.ndarray, offset: np.ndarray) -> np.ndarray:
    global _LAST_RESULT, _NC_CACHE
    assert x.shape == (B_FULL, C, H, W), x.shape
    ops1, ops2 = _build_ops(np.asarray(offset, dtype=np.float32))
    if _NC_CACHE is None:
        _NC_CACHE = _build_bass()
    nc = _NC_CACHE

    # host pack: fp16 cast + [p, (c, x)] layout; batch 2bp rows on
    # partitions 0:64, batch 2bp+1 rows on 64:128 (index permutation only).
    x16 = np.asarray(x, dtype=np.float32).astype(np.float16)
    xv = x16.reshape(N_CORES, N_BPAIR, 2, C, H, W)
    xP = np.empty((N_CORES, N_BPAIR, 128, C, W), dtype=np.float16)
    xP[:, :, 0:64] = xv[:, :, 0].transpose(0, 1, 3, 2, 4)   # [i,bp,y,c,x]
    xP[:, :, 64:128] = xv[:, :, 1].transpose(0, 1, 3, 2, 4)
    xP = xP.reshape(N_CORES, N_BPAIR, 128, XCOLS)

    in_maps = []
    for i in range(N_CORES):
        in_maps.append({"x": xP[i], "ops1": ops1, "ops2": ops2})
    res = run_bass_kernel_spmd(nc, in_maps, list(range(N_CORES)))
    _LAST_RESULT = res

    # host unpack: y[i] [bp, (u', yy), (g, m, s, x)];
    # channel = 8g + 2m + u', batch = 4i + 2bp + s.
    out = np.empty((B_FULL, C, H, W), dtype=np.float32)
    for i in range(N_CORES):
        yv = res.results[i]["y"].reshape(N_BPAIR, 2, H, P_POS, GROUP // 2,
                                         2, W)
        yt = yv.transpose(0, 5, 3, 4, 1, 2, 6)   # bp s g m u' yy x
        out[4 * i:4 * i + 4] = yt.reshape(B_LOC, C, H, W)
    return out


if __name__ == "__main__":
    nc = _build_bass()
    print("bass program built ok")


# revision 24
# speedup vs baseline: 1.0914x; 1.0914x over previous
"""DisplaceChannel (integer displace + per-position 5x5 gaussian depthwise
conv) as a Bass/Tile kernel for 8 Trainium2 NeuronCores.

Math: the 5x5 gaussian kernel is separable and its normalizer factorizes;
the integer shift + 'same' zero-padding fold into banded 64x64 row/col
operators built host-side from the tiny (48,2) `offset`.  Per image:

    out = Vy @ X @ Vx^T

On device each channel-pair of images (same channel, adjacent batches)
runs two chained PE matmuls (fp16 operands, fp32 PSUM accumulate):

  pass1: two concurrent 64x64 quadrant matmuls (tile_position (0,0) and
         (64,64)): lhsT = image rows (stationary), rhs = [R1; R1] slices,
         R1 = Vy^T -> PSUM [128,64] = [A^T R1 ; B^T R1]  ([x', y] each)
  pass2: lhsT = two pass1 pair-results side by side [128,128] (cast fp16),
         rhs = blockdiag(R2, R2) [128,128], R2 = Vx^T
         -> PSUM [128,128]: partitions = (channel-sub, y), cols = (batch, x)

Sharding: data-parallel over batch (4 per core); operators replicated.

I/O layout: the host packs the input (fp16 cast + index permutation) and
unpacks the output so that every device DMA moves contiguous 12KB runs
per partition.  In the natural NCHW layout each (channel,row) is a
256-byte descriptor and the kernel is SDMA-packet-rate-bound; packed, it
runs at the HBM byte roofline.
"""

import numpy as np

from concourse import bacc, mybir, tile
from concourse.bass_utils import run_bass_kernel_spmd

# problem constants (hardcoded per harness contract)
B_FULL, C, H, W = 32, 384, 64, 64
N_CORES = 8
B_LOC = B_FULL // N_CORES          # 4 batches per core
P_POS = 48                         # offset positions; C // P_POS = 8 chan/pos
GROUP = C // P_POS                 # 8 channels share one operator pair
KSZ, SIGMA, CK = 5, 0.5, 2

N_BPAIR = B_LOC // 2               # batch-pairs (2bp, 2bp+1) per core
IN_GROUPS = 12                     # groups per input chunk (96 channels)
OUT_GROUPS = 6                     # groups per output chunk (48 channels)
IN_COLS = IN_GROUPS * GROUP * 64   # 6144 fp16 cols per in-chunk
OUT_COLS = OUT_GROUPS * GROUP * 64  # 3072 fp32 cols per out-chunk
XCOLS = C * 64                     # 24576 per-bp packed cols

FP16 = mybir.dt.float16
FP32 = mybir.dt.float32

_LAST_RESULT = None                # test.py introspection (profile/exec time)


def _shift_conv_matrix(sub, d):
    """[64(src), 64(out)] with R[src,out] = k[i], src = out + i - 2 - d,
    masked by conv zero-pad (0<=out+i-2<64) and shift zero-fill (0<=src<64)."""
    k = np.exp(-((np.arange(KSZ) - CK + sub) ** 2) / (2.0 * SIGMA**2))
    k = k / k.sum()
    R = np.zeros((H, H), dtype=np.float64)
    out = np.arange(H)
    for i in range(KSZ):
        t = out + i - CK            # coordinate in the shifted image
        src = t - d
        m = (t >= 0) & (t < H) & (src >= 0) & (src < H)
        R[src[m], out[m]] += k[i]
    return R


def _build_ops(offset):
    """ops1 [128, 48*64] fp16 : per position the stacked [R1; R1] (R1 = Vy^T)
    ops2 [128, 48*128] fp16 : per position blockdiag(R2, R2)   (R2 = Vx^T)"""
    off_round = np.round(offset.astype(np.float64))
    off_int = off_round.astype(np.int64)
    sub = offset.astype(np.float64) - off_round
    ops1 = np.zeros((128, P_POS * 64), dtype=np.float64)
    ops2 = np.zeros((128, P_POS * 128), dtype=np.float64)
    for p in range(P_POS):
        R1 = _shift_conv_matrix(sub[p, 1], off_int[p, 1])   # y: suby, dy
        R2 = _shift_conv_matrix(sub[p, 0], off_int[p, 0])   # x: subx, dx
        ops1[0:64, 64 * p:64 * p + 64] = R1
        ops1[64:128, 64 * p:64 * p + 64] = R1
        ops2[0:64, 128 * p:128 * p + 64] = R2
        ops2[64:128, 128 * p + 64:128 * p + 128] = R2
    return ops1.astype(np.float16), ops2.astype(np.float16)


def _build_bass():
    nc = bacc.Bacc(
        "TRN2",
        target_bir_lowering=False,
        debug=False,
        num_devices=N_CORES,
    )
    # packed fp16 input: per bp a [128, 24576] block; channel c at cols
    # 64c:64c+64, batch 2bp rows on partitions 0:64, batch 2bp+1 on 64:128.
    x_in = nc.declare_dram_parameter("x", [N_BPAIR, 128, XCOLS], FP16,
                                     isOutput=False)
    ops1_in = nc.declare_dram_parameter("ops1", [128, P_POS * 64], FP16,
                                        isOutput=False)
    ops2_in = nc.declare_dram_parameter("ops2", [128, P_POS * 128], FP16,
                                        isOutput=False)
    # packed output: per bp [128, 24576] f32; cols (g, m, s, x), partitions
    # (u', y); channel = 8g + 2m + u', batch = 2bp + s.
    y_out = nc.declare_dram_parameter("y", [N_BPAIR, 128, XCOLS], FP32,
                                      isOutput=True)

    with tile.TileContext(nc) as tc:
        with (
            tc.tile_pool(name="consts", bufs=1) as consts,
            tc.tile_pool(name="wchunk", bufs=4) as wpool,
            tc.tile_pool(name="l2", bufs=4) as l2pool,
            tc.tile_pool(name="outs", bufs=3) as outpool,
            tc.tile_pool(name="psum1", bufs=3, space="PSUM") as psum1p,
            tc.tile_pool(name="psum2", bufs=3, space="PSUM") as psum2p,
        ):
            t_ops1 = consts.tile([128, P_POS * 64], FP16)
            t_ops2 = consts.tile([128, P_POS * 128], FP16)
            nc.sync.dma_start(out=t_ops1[:], in_=ops1_in[:])
            nc.sync.dma_start(out=t_ops2[:], in_=ops2_in[:])

            # flattened (bp, group) stream with lookahead in-DMA emission
            # (SWDGE executes its queue in emission order, so chunk k+2's
            # DMA is emitted before chunk k's compute).
            chunk_list = []                     # (bp, ga, gb)
            for bp in range(N_BPAIR):
                starts = ([0, 2, 6] if bp == 0 else [0]) + \
                    list(range(12, P_POS, 12))
                for ci, a in enumerate(starts):
                    b = starts[ci + 1] if ci + 1 < len(starts) else P_POS
                    chunk_list.append((bp, a, b))
            g2chunk = {}
            for ci, (bp, a, b) in enumerate(chunk_list):
                for g in range(a, b):
                    g2chunk[(bp, g)] = ci
            LOOKAHEAD = 2
            wts = {}
            n_emitted = 0

            def emit_in_dma(ci):
                bp, a, b = chunk_list[ci]
                wt = wpool.tile([128, (b - a) * GROUP * 64], FP16,
                                tag=f"w{b - a}")
                nc.gpsimd.dma_start(
                    out=wt[:],
                    in_=x_in[bp][:, a * GROUP * 64:b * GROUP * 64])
                wts[ci] = wt

            it = 0
            outs = None
            for bp in range(N_BPAIR):
                for g in range(P_POS):                      # 48 groups / bp
                    ci = g2chunk[(bp, g)]
                    while n_emitted <= min(ci + LOOKAHEAD, len(chunk_list) - 1):
                        emit_in_dma(n_emitted)
                        n_emitted += 1
                    if g % OUT_GROUPS == 0:
                        outs = outpool.tile([128, OUT_COLS], FP32)
                    wt = wts[ci]
                    goff = g - chunk_list[ci][1]     # group offset in chunk

                    rhs1 = t_ops1[:, 64 * g:64 * g + 64]
                    rhs2 = t_ops2[:, 128 * g:128 * g + 128]
                    ps1 = psum1p.tile([128, 512], FP32)
                    ps2 = psum2p.tile([128, 512], FP32)
                    l2 = l2pool.tile([128, 512], FP16)

                    for j in range(GROUP):                   # 8 pairs
                        jc = goff * GROUP + j                # chan in chunk
                        cs = slice(64 * jc, 64 * jc + 64)
                        # A (batch 2bp): quadrant rows 0:63 x cols 0:63
                        nc.tensor.matmul(ps1[0:64, 64 * j:64 * j + 64],
                                         wt[0:64, cs], rhs1[0:64, :],
                                         start=True, stop=True,
                                         tile_position=(0, 0))
                        # B (batch 2bp+1): quadrant rows 64:127 x 64:127
                        nc.tensor.matmul(ps1[64:128, 64 * j:64 * j + 64],
                                         wt[64:128, cs], rhs1[64:128, :],
                                         start=True, stop=True,
                                         tile_position=(64, 64))
                    # psum fp32 -> sbuf fp16 (pass2 stationary operand)
                    if it % 2 == 0:
                        nc.vector.tensor_copy(l2[:], ps1[:])
                    else:
                        nc.scalar.copy(l2[:], ps1[:])
                    for m in range(GROUP // 2):              # 4 two-pair MMs
                        lhsT2 = l2[:, 128 * m:128 * m + 128]
                        nc.tensor.matmul(ps2[:, 128 * m:128 * m + 128],
                                         lhsT2, rhs2,
                                         start=True, stop=True)
                    # final psum -> staging (fp32, full partitions)
                    od = outs[:, 512 * (g % OUT_GROUPS):
                              512 * (g % OUT_GROUPS) + 512]
                    if it % 2 == 0:
                        nc.scalar.copy(od, ps2[:])
                    else:
                        nc.vector.tensor_copy(od, ps2[:])
                    it += 1

                    if g % OUT_GROUPS == OUT_GROUPS - 1:
                        oc = g // OUT_GROUPS
                        nc.sync.dma_start(
                            out=y_out[bp][:, OUT_COLS * oc:OUT_COLS * (oc + 1)],
                            in_=outs[:])
    nc.compile()
    return nc


_NC_CACHE = None


def kernel(x: np.ndarray, offset: np.ndarray) -> np.ndarray:
    global _LAST_RESULT, _NC_CACHE
    assert x.shape == (B_FULL, C, H, W), x.shape
    ops1, ops2 = _build_ops(np.asarray(offset, dtype=np.float32))
    if _NC_CACHE is None:
        _NC_CACHE = _build_bass()
    nc = _NC_CACHE

    # host pack: fp16 cast + [p, (c, x)] layout; batch 2bp rows on
    # partitions 0:64, batch 2bp+1 rows on 64:128 (index permutation only).
    x16 = np.asarray(x, dtype=np.float32).astype(np.float16)
    xv = x16.reshape(N_CORES, N_BPAIR, 2, C, H, W)
    xP = np.empty((N_CORES, N_BPAIR, 128, C, W), dtype=np.float16)
    xP[:, :, 0:64] = xv[:, :, 0].transpose(0, 1, 3, 2, 4)   # [i,bp,y,c,x]
    xP[:, :, 64:128] = xv[:, :, 1].transpose(0, 1, 3, 2, 4)
    xP = xP.reshape(N_CORES, N_BPAIR, 128, XCOLS)

    in_maps = []
    for i in range(N_CORES):
        in_maps.append({"x": xP[i], "ops1": ops1, "ops2": ops2})
    res = run_bass_kernel_spmd(nc, in_maps, list(range(N_CORES)))
    _LAST_RESULT = res

    # host unpack: y[i] [bp, (u', yy), (g, m, s, x)];
    # channel = 8g + 2m + u', batch = 4i + 2bp + s.
    out = np.empty((B_FULL, C, H, W), dtype=np.float32)
    for i in range(N_CORES):
        yv = res.results[i]["y"].reshape(N_BPAIR, 2, H, P_POS, GROUP // 2,
                                         2, W)
        yt = yv.transpose(0, 5, 3, 4, 1, 2, 6)   # bp s g m u' yy x
        out[4 * i:4 * i + 4] = yt.reshape(B_LOC, C, H, W)
    return out


if __name__ == "__main__":
    nc = _build_bass()
    print("bass program built ok")


# revision 25
# speedup vs baseline: 1.2168x; 1.1148x over previous
"""DisplaceChannel (integer displace + per-position 5x5 gaussian depthwise
conv) as a Bass/Tile kernel for 8 Trainium2 NeuronCores.

Math: the 5x5 gaussian kernel is separable and its normalizer factorizes;
the integer shift + 'same' zero-padding fold into banded 64x64 row/col
operators built host-side from the tiny (48,2) `offset`.  Per image:

    out = Vy @ X @ Vx^T

On device each channel-pair of images (same channel, adjacent batches)
runs two chained PE matmuls (fp16 operands, fp32 PSUM accumulate):

  pass1: two concurrent 64x64 quadrant matmuls (tile_position (0,0) and
         (64,64)): lhsT = image rows (stationary), rhs = [R1; R1] slices,
         R1 = Vy^T -> PSUM [128,64] = [A^T R1 ; B^T R1]  ([x', y] each)
  pass2: lhsT = two pass1 pair-results side by side [128,128] (cast fp16),
         rhs = blockdiag(R2, R2) [128,128], R2 = Vx^T
         -> PSUM [128,128]: partitions = (channel-sub, y), cols = (batch, x)

Sharding: data-parallel over batch (4 per core); operators replicated.

I/O layout: the host packs the input (fp16 cast + index permutation) and
unpacks the output so that every device DMA moves contiguous 12KB runs
per partition.  In the natural NCHW layout each (channel,row) is a
256-byte descriptor and the kernel is SDMA-packet-rate-bound; packed, it
runs at the HBM byte roofline.
"""

import numpy as np

from concourse import bacc, mybir, tile
from concourse.bass_utils import run_bass_kernel_spmd

# problem constants (hardcoded per harness contract)
B_FULL, C, H, W = 32, 384, 64, 64
N_CORES = 8
B_LOC = B_FULL // N_CORES          # 4 batches per core
P_POS = 48                         # offset positions; C // P_POS = 8 chan/pos
GROUP = C // P_POS                 # 8 channels share one operator pair
KSZ, SIGMA, CK = 5, 0.5, 2

N_BPAIR = B_LOC // 2               # batch-pairs (2bp, 2bp+1) per core
IN_GROUPS = 12                     # groups per input chunk (96 channels)
OUT_GROUPS = 6                     # groups per output chunk (48 channels)
IN_COLS = IN_GROUPS * GROUP * 64   # 6144 fp16 cols per in-chunk
OUT_COLS = OUT_GROUPS * GROUP * 64  # 3072 fp32 cols per out-chunk
XCOLS = C * 64                     # 24576 per-bp packed cols

FP16 = mybir.dt.float16
FP32 = mybir.dt.float32

_LAST_RESULT = None                # test.py introspection (profile/exec time)


def _shift_conv_matrix(sub, d):
    """[64(src), 64(out)] with R[src,out] = k[i], src = out + i - 2 - d,
    masked by conv zero-pad (0<=out+i-2<64) and shift zero-fill (0<=src<64)."""
    k = np.exp(-((np.arange(KSZ) - CK + sub) ** 2) / (2.0 * SIGMA**2))
    k = k / k.sum()
    R = np.zeros((H, H), dtype=np.float64)
    out = np.arange(H)
    for i in range(KSZ):
        t = out + i - CK            # coordinate in the shifted image
        src = t - d
        m = (t >= 0) & (t < H) & (src >= 0) & (src < H)
        R[src[m], out[m]] += k[i]
    return R


def _build_ops(offset):
    """ops1 [128, 48*64] fp16 : per position the stacked [R1; R1] (R1 = Vy^T)
    ops2 [128, 48*128] fp16 : per position blockdiag(R2, R2)   (R2 = Vx^T)"""
    off_round = np.round(offset.astype(np.float64))
    off_int = off_round.astype(np.int64)
    sub = offset.astype(np.float64) - off_round
    ops1 = np.zeros((128, P_POS * 64), dtype=np.float64)
    ops2 = np.zeros((128, P_POS * 128), dtype=np.float64)
    for p in range(P_POS):
        R1 = _shift_conv_matrix(sub[p, 1], off_int[p, 1])   # y: suby, dy
        R2 = _shift_conv_matrix(sub[p, 0], off_int[p, 0])   # x: subx, dx
        ops1[0:64, 64 * p:64 * p + 64] = R1
        ops1[64:128, 64 * p:64 * p + 64] = R1
        ops2[0:64, 128 * p:128 * p + 64] = R2
        ops2[64:128, 128 * p + 64:128 * p + 128] = R2
    return ops1.astype(np.float16), ops2.astype(np.float16)


def _build_bass():
    nc = bacc.Bacc(
        "TRN2",
        target_bir_lowering=False,
        debug=False,
        num_devices=N_CORES,
    )
    # packed fp16 input: per bp a [128, 24576] block; channel c at cols
    # 64c:64c+64, batch 2bp rows on partitions 0:64, batch 2bp+1 on 64:128.
    x_in = nc.declare_dram_parameter("x", [N_BPAIR, 128, XCOLS], FP16,
                                     isOutput=False)
    ops1_in = nc.declare_dram_parameter("ops1", [128, P_POS * 64], FP16,
                                        isOutput=False)
    ops2_in = nc.declare_dram_parameter("ops2", [128, P_POS * 128], FP16,
                                        isOutput=False)
    # packed output: per bp [128, 24576] f32; cols (g, m, s, x), partitions
    # (u', y); channel = 8g + 2m + u', batch = 2bp + s.
    y_out = nc.declare_dram_parameter("y", [N_BPAIR, 128, XCOLS], FP16,
                                      isOutput=True)

    with tile.TileContext(nc) as tc:
        with (
            tc.tile_pool(name="consts", bufs=1) as consts,
            tc.tile_pool(name="wchunk", bufs=4) as wpool,
            tc.tile_pool(name="l2", bufs=4) as l2pool,
            tc.tile_pool(name="outs", bufs=3) as outpool,
            tc.tile_pool(name="psum1", bufs=3, space="PSUM") as psum1p,
            tc.tile_pool(name="psum2", bufs=3, space="PSUM") as psum2p,
        ):
            t_ops1 = consts.tile([128, P_POS * 64], FP16)
            t_ops2 = consts.tile([128, P_POS * 128], FP16)
            nc.sync.dma_start(out=t_ops1[:], in_=ops1_in[:])
            nc.sync.dma_start(out=t_ops2[:], in_=ops2_in[:])

            # flattened (bp, group) stream with lookahead in-DMA emission
            # (SWDGE executes its queue in emission order, so chunk k+2's
            # DMA is emitted before chunk k's compute).
            chunk_list = []                     # (bp, ga, gb)
            for bp in range(N_BPAIR):
                starts = ([0, 2, 6] if bp == 0 else [0]) + \
                    list(range(12, P_POS, 12))
                for ci, a in enumerate(starts):
                    b = starts[ci + 1] if ci + 1 < len(starts) else P_POS
                    chunk_list.append((bp, a, b))
            g2chunk = {}
            for ci, (bp, a, b) in enumerate(chunk_list):
                for g in range(a, b):
                    g2chunk[(bp, g)] = ci
            LOOKAHEAD = 2
            wts = {}
            n_emitted = 0

            def emit_in_dma(ci):
                bp, a, b = chunk_list[ci]
                wt = wpool.tile([128, (b - a) * GROUP * 64], FP16,
                                tag=f"w{b - a}")
                nc.gpsimd.dma_start(
                    out=wt[:],
                    in_=x_in[bp][:, a * GROUP * 64:b * GROUP * 64])
                wts[ci] = wt

            it = 0
            outs = None
            for bp in range(N_BPAIR):
                for g in range(P_POS):                      # 48 groups / bp
                    ci = g2chunk[(bp, g)]
                    while n_emitted <= min(ci + LOOKAHEAD, len(chunk_list) - 1):
                        emit_in_dma(n_emitted)
                        n_emitted += 1
                    if g % OUT_GROUPS == 0:
                        outs = outpool.tile([128, OUT_COLS], FP16)
                    wt = wts[ci]
                    goff = g - chunk_list[ci][1]     # group offset in chunk

                    rhs1 = t_ops1[:, 64 * g:64 * g + 64]
                    rhs2 = t_ops2[:, 128 * g:128 * g + 128]
                    ps1 = psum1p.tile([128, 512], FP32)
                    ps2 = psum2p.tile([128, 512], FP32)
                    l2 = l2pool.tile([128, 512], FP16)

                    for j in range(GROUP):                   # 8 pairs
                        jc = goff * GROUP + j                # chan in chunk
                        cs = slice(64 * jc, 64 * jc + 64)
                        # A (batch 2bp): quadrant rows 0:63 x cols 0:63
                        nc.tensor.matmul(ps1[0:64, 64 * j:64 * j + 64],
                                         wt[0:64, cs], rhs1[0:64, :],
                                         start=True, stop=True,
                                         tile_position=(0, 0))
                        # B (batch 2bp+1): quadrant rows 64:127 x 64:127
                        nc.tensor.matmul(ps1[64:128, 64 * j:64 * j + 64],
                                         wt[64:128, cs], rhs1[64:128, :],
                                         start=True, stop=True,
                                         tile_position=(64, 64))
                    # psum fp32 -> sbuf fp16 (pass2 stationary operand)
                    if it % 2 == 0:
                        nc.vector.tensor_copy(l2[:], ps1[:])
                    else:
                        nc.scalar.copy(l2[:], ps1[:])
                    for m in range(GROUP // 2):              # 4 two-pair MMs
                        lhsT2 = l2[:, 128 * m:128 * m + 128]
                        nc.tensor.matmul(ps2[:, 128 * m:128 * m + 128],
                                         lhsT2, rhs2,
                                         start=True, stop=True)
                    # final psum -> staging (fp32, full partitions)
                    od = outs[:, 512 * (g % OUT_GROUPS):
                              512 * (g % OUT_GROUPS) + 512]
                    if it % 2 == 0:
                        nc.scalar.copy(od, ps2[:])
                    else:
                        nc.vector.tensor_copy(od, ps2[:])
                    it += 1

                    if g % OUT_GROUPS == OUT_GROUPS - 1:
                        oc = g // OUT_GROUPS
                        nc.sync.dma_start(
                            out=y_out[bp][:, OUT_COLS * oc:OUT_COLS * (oc + 1)],
                            in_=outs[:])
    nc.compile()
    return nc


_NC_CACHE = None


def kernel(x: np.ndarray, offset: np.ndarray) -> np.ndarray:
    global _LAST_RESULT, _NC_CACHE
    assert x.shape == (B_FULL, C, H, W), x.shape
    ops1, ops2 = _build_ops(np.asarray(offset, dtype=np.float32))
    if _NC_CACHE is None:
        _NC_CACHE = _build_bass()
    nc = _NC_CACHE

    # host pack: fp16 cast + [p, (c, x)] layout; batch 2bp rows on
    # partitions 0:64, batch 2bp+1 rows on 64:128 (index permutation only).
    x16 = np.asarray(x, dtype=np.float32).astype(np.float16)
    xv = x16.reshape(N_CORES, N_BPAIR, 2, C, H, W)
    xP = np.empty((N_CORES, N_BPAIR, 128, C, W), dtype=np.float16)
    xP[:, :, 0:64] = xv[:, :, 0].transpose(0, 1, 3, 2, 4)   # [i,bp,y,c,x]
    xP[:, :, 64:128] = xv[:, :, 1].transpose(0, 1, 3, 2, 4)
    xP = xP.reshape(N_CORES, N_BPAIR, 128, XCOLS)

    in_maps = []
    for i in range(N_CORES):
        in_maps.append({"x": xP[i], "ops1": ops1, "ops2": ops2})
    res = run_bass_kernel_spmd(nc, in_maps, list(range(N_CORES)))
    _LAST_RESULT = res

    # host unpack: y[i] [bp, (u', yy), (g, m, s, x)];
    # channel = 8g + 2m + u', batch = 4i + 2bp + s.
    out = np.empty((B_FULL, C, H, W), dtype=np.float32)
    for i in range(N_CORES):
        yv = res.results[i]["y"].astype(np.float32).reshape(
            N_BPAIR, 2, H, P_POS, GROUP // 2, 2, W)
        yt = yv.transpose(0, 5, 3, 4, 1, 2, 6)   # bp s g m u' yy x
        out[4 * i:4 * i + 4] = yt.reshape(B_LOC, C, H, W)
    return out


if __name__ == "__main__":
    nc = _build_bass()
    print("bass program built ok")


# revision 26
# speedup vs baseline: 1.2357x; 1.0156x over previous
"""DisplaceChannel (integer displace + per-position 5x5 gaussian depthwise
conv) as a Bass/Tile kernel for 8 Trainium2 NeuronCores.

Math: the 5x5 gaussian kernel is separable and its normalizer factorizes;
the integer shift + 'same' zero-padding fold into banded 64x64 row/col
operators built host-side from the tiny (48,2) `offset`.  Per image:

    out = Vy @ X @ Vx^T

On device each channel-pair of images (same channel, adjacent batches)
runs two chained PE matmuls (fp16 operands, fp32 PSUM accumulate):

  pass1: two concurrent 64x64 quadrant matmuls (tile_position (0,0) and
         (64,64)): lhsT = image rows (stationary), rhs = [R1; R1] slices,
         R1 = Vy^T -> PSUM [128,64] = [A^T R1 ; B^T R1]  ([x', y] each)
  pass2: lhsT = two pass1 pair-results side by side [128,128] (cast fp16),
         rhs = blockdiag(R2, R2) [128,128], R2 = Vx^T
         -> PSUM [128,128]: partitions = (channel-sub, y), cols = (batch, x)

Sharding: data-parallel over batch (4 per core); operators replicated.

I/O layout: the host packs the input (fp16 cast + index permutation) and
unpacks the output so that every device DMA moves contiguous 12KB runs
per partition.  In the natural NCHW layout each (channel,row) is a
256-byte descriptor and the kernel is SDMA-packet-rate-bound; packed, it
runs at the HBM byte roofline.
"""

import numpy as np

from concourse import bacc, mybir, tile
from concourse.bass_utils import run_bass_kernel_spmd

# problem constants (hardcoded per harness contract)
B_FULL, C, H, W = 32, 384, 64, 64
N_CORES = 8
B_LOC = B_FULL // N_CORES          # 4 batches per core
P_POS = 48                         # offset positions; C // P_POS = 8 chan/pos
GROUP = C // P_POS                 # 8 channels share one operator pair
KSZ, SIGMA, CK = 5, 0.5, 2

N_BPAIR = B_LOC // 2               # batch-pairs (2bp, 2bp+1) per core
IN_GROUPS = 12                     # groups per input chunk (96 channels)
OUT_GROUPS = 6                     # groups per output chunk (48 channels)
IN_COLS = IN_GROUPS * GROUP * 64   # 6144 fp16 cols per in-chunk
OUT_COLS = OUT_GROUPS * GROUP * 64  # 3072 cols per out-chunk
XCOLS = C * 64                     # 24576 per-bp packed cols

FP16 = mybir.dt.float16
FP32 = mybir.dt.float32

_LAST_RESULT = None                # test.py introspection (profile/exec time)


def _shift_conv_matrix(sub, d):
    """[64(src), 64(out)] with R[src,out] = k[i], src = out + i - 2 - d,
    masked by conv zero-pad (0<=out+i-2<64) and shift zero-fill (0<=src<64)."""
    k = np.exp(-((np.arange(KSZ) - CK + sub) ** 2) / (2.0 * SIGMA**2))
    k = k / k.sum()
    R = np.zeros((H, H), dtype=np.float64)
    out = np.arange(H)
    for i in range(KSZ):
        t = out + i - CK            # coordinate in the shifted image
        src = t - d
        m = (t >= 0) & (t < H) & (src >= 0) & (src < H)
        R[src[m], out[m]] += k[i]
    return R


def _build_ops(offset):
    """ops1 [128, 48*64] fp16 : per position the stacked [R1; R1] (R1 = Vy^T)
    ops2 [128, 48*128] fp16 : per position blockdiag(R2, R2)   (R2 = Vx^T)"""
    off_round = np.round(offset.astype(np.float64))
    off_int = off_round.astype(np.int64)
    sub = offset.astype(np.float64) - off_round
    ops1 = np.zeros((128, P_POS * 64), dtype=np.float64)
    ops2 = np.zeros((128, P_POS * 128), dtype=np.float64)
    for p in range(P_POS):
        R1 = _shift_conv_matrix(sub[p, 1], off_int[p, 1])   # y: suby, dy
        R2 = _shift_conv_matrix(sub[p, 0], off_int[p, 0])   # x: subx, dx
        ops1[0:64, 64 * p:64 * p + 64] = R1
        ops1[64:128, 64 * p:64 * p + 64] = R1
        ops2[0:64, 128 * p:128 * p + 64] = R2
        ops2[64:128, 128 * p + 64:128 * p + 128] = R2
    return ops1.astype(np.float16), ops2.astype(np.float16)


def _build_bass():
    nc = bacc.Bacc(
        "TRN2",
        target_bir_lowering=False,
        debug=False,
        num_devices=N_CORES,
    )
    # packed fp16 input: per bp a [128, 24576] block; channel c at cols
    # 64c:64c+64, batch 2bp rows on partitions 0:64, batch 2bp+1 on 64:128.
    x_in = nc.declare_dram_parameter("x", [N_BPAIR, 128, XCOLS], FP16,
                                     isOutput=False)
    ops1_in = nc.declare_dram_parameter("ops1", [128, P_POS * 64], FP16,
                                        isOutput=False)
    ops2_in = nc.declare_dram_parameter("ops2", [128, P_POS * 128], FP16,
                                        isOutput=False)
    # packed output: per bp [128, 24576] fp16 (host upcasts to f32);
    # cols (g, m, s, x), partitions
    # (u', y); channel = 8g + 2m + u', batch = 2bp + s.
    y_out = nc.declare_dram_parameter("y", [N_BPAIR, 128, XCOLS], FP16,
                                      isOutput=True)

    with tile.TileContext(nc) as tc:
        with (
            tc.tile_pool(name="consts", bufs=1) as consts,
            tc.tile_pool(name="wchunk", bufs=4) as wpool,
            tc.tile_pool(name="l2", bufs=4) as l2pool,
            tc.tile_pool(name="outs", bufs=3) as outpool,
            tc.tile_pool(name="psum1", bufs=3, space="PSUM") as psum1p,
            tc.tile_pool(name="psum2", bufs=3, space="PSUM") as psum2p,
        ):
            t_ops1 = consts.tile([128, P_POS * 64], FP16)
            t_ops2 = consts.tile([128, P_POS * 128], FP16)
            nc.sync.dma_start(out=t_ops1[:], in_=ops1_in[:])
            nc.sync.dma_start(out=t_ops2[:], in_=ops2_in[:])

            # flattened (bp, group) stream with lookahead in-DMA emission
            # (SWDGE executes its queue in emission order, so chunk k+2's
            # DMA is emitted before chunk k's compute).
            chunk_list = []                     # (bp, ga, gb)
            for bp in range(N_BPAIR):
                starts = ([0, 2, 6] if bp == 0 else [0]) + \
                    list(range(12, P_POS, 12))
                for ci, a in enumerate(starts):
                    b = starts[ci + 1] if ci + 1 < len(starts) else P_POS
                    chunk_list.append((bp, a, b))
            g2chunk = {}
            for ci, (bp, a, b) in enumerate(chunk_list):
                for g in range(a, b):
                    g2chunk[(bp, g)] = ci
            LOOKAHEAD = 2
            wts = {}
            n_emitted = 0

            def emit_in_dma(ci):
                bp, a, b = chunk_list[ci]
                wt = wpool.tile([128, (b - a) * GROUP * 64], FP16,
                                tag=f"w{b - a}")
                nc.gpsimd.dma_start(
                    out=wt[:],
                    in_=x_in[bp][:, a * GROUP * 64:b * GROUP * 64])
                wts[ci] = wt

            it = 0
            outs = None
            for bp in range(N_BPAIR):
                for g in range(P_POS):                      # 48 groups / bp
                    ci = g2chunk[(bp, g)]
                    while n_emitted <= min(ci + LOOKAHEAD, len(chunk_list) - 1):
                        emit_in_dma(n_emitted)
                        n_emitted += 1
                    if g % OUT_GROUPS == 0:
                        outs = outpool.tile([128, OUT_COLS], FP16)
                    wt = wts[ci]
                    goff = g - chunk_list[ci][1]     # group offset in chunk

                    rhs1 = t_ops1[:, 64 * g:64 * g + 64]
                    rhs2 = t_ops2[:, 128 * g:128 * g + 128]
                    ps1 = psum1p.tile([128, 512], FP32)
                    ps2 = psum2p.tile([128, 512], FP32)
                    l2 = l2pool.tile([128, 512], FP16)

                    for j in range(GROUP):                   # 8 pairs
                        jc = goff * GROUP + j                # chan in chunk
                        cs = slice(64 * jc, 64 * jc + 64)
                        # A (batch 2bp): quadrant rows 0:63 x cols 0:63
                        nc.tensor.matmul(ps1[0:64, 64 * j:64 * j + 64],
                                         wt[0:64, cs], rhs1[0:64, :],
                                         start=True, stop=True,
                                         tile_position=(0, 0))
                        # B (batch 2bp+1): quadrant rows 64:127 x 64:127
                        nc.tensor.matmul(ps1[64:128, 64 * j:64 * j + 64],
                                         wt[64:128, cs], rhs1[64:128, :],
                                         start=True, stop=True,
                                         tile_position=(64, 64))
                    # psum fp32 -> sbuf fp16 (pass2 stationary operand)
                    if it % 2 == 0:
                        nc.vector.tensor_copy(l2[:], ps1[:])
                    else:
                        nc.scalar.copy(l2[:], ps1[:])
                    for m in range(GROUP // 2):              # 4 two-pair MMs
                        lhsT2 = l2[:, 128 * m:128 * m + 128]
                        nc.tensor.matmul(ps2[:, 128 * m:128 * m + 128],
                                         lhsT2, rhs2,
                                         start=True, stop=True)
                    # final psum -> staging (fp16, full partitions)
                    od = outs[:, 512 * (g % OUT_GROUPS):
                              512 * (g % OUT_GROUPS) + 512]
                    if it % 2 == 0:
                        nc.scalar.copy(od, ps2[:])
                    else:
                        nc.vector.tensor_copy(od, ps2[:])
                    it += 1

                    if g % OUT_GROUPS == OUT_GROUPS - 1:
                        oc = g // OUT_GROUPS
                        nc.sync.dma_start(
                            out=y_out[bp][:, OUT_COLS * oc:OUT_COLS * (oc + 1)],
                            in_=outs[:])
    nc.compile()
    return nc


_NC_CACHE = None


def kernel(x: np.ndarray, offset: np.ndarray) -> np.ndarray:
    global _LAST_RESULT, _NC_CACHE
    assert x.shape == (B_FULL, C, H, W), x.shape
    ops1, ops2 = _build_ops(np.asarray(offset, dtype=np.float32))
    if _NC_CACHE is None:
        _NC_CACHE = _build_bass()
    nc = _NC_CACHE

    # host pack: fp16 cast + [p, (c, x)] layout; batch 2bp rows on
    # partitions 0:64, batch 2bp+1 rows on 64:128 (index permutation only).
    x16 = np.asarray(x, dtype=np.float32).astype(np.float16)
    xv = x16.reshape(N_CORES, N_BPAIR, 2, C, H, W)
    xP = np.empty((N_CORES, N_BPAIR, 128, C, W), dtype=np.float16)
    xP[:, :, 0:64] = xv[:, :, 0].transpose(0, 1, 3, 2, 4)   # [i,bp,y,c,x]
    xP[:, :, 64:128] = xv[:, :, 1].transpose(0, 1, 3, 2, 4)
    xP = xP.reshape(N_CORES, N_BPAIR, 128, XCOLS)

    in_maps = []
    for i in range(N_CORES):
        in_maps.append({"x": xP[i], "ops1": ops1, "ops2": ops2})
    res = run_bass_kernel_spmd(nc, in_maps, list(range(N_CORES)))
    _LAST_RESULT = res

    # host unpack: y[i] [bp, (u', yy), (g, m, s, x)];
    # channel = 8g + 2m + u', batch = 4i + 2bp + s.
    out = np.empty((B_FULL, C, H, W), dtype=np.float32)
    for i in range(N_CORES):
        yv = res.results[i]["y"].astype(np.float32).reshape(
            N_BPAIR, 2, H, P_POS, GROUP // 2, 2, W)
        yt = yv.transpose(0, 5, 3, 4, 1, 2, 6)   # bp s g m u' yy x
        out[4 * i:4 * i + 4] = yt.reshape(B_LOC, C, H, W)
    return out


if __name__ == "__main__":
    nc = _build_bass()
    print("bass program built ok")
